# revision 1
# baseline (speedup 1.0000x reference)
"""Trainium2 Bass kernel for ClassLinearWithLORA (moe_routing).

Computes out = x @ W.T + b + gates[-1] * (alpha * (x @ A[-1]) @ B_lora[-1])
(the torch loop overwrites out_lora each class iteration, so only the last
class adapter contributes).

Strategy:
  - Data-parallel shard of the 8192 (B*S) rows across 8 NeuronCores
    (1024 rows/core); W/b and the rank-16 LoRA stacks are replicated.
  - Matmuls run as fp32r (fp32 with mantissa rounded to 11 bits, TF32-like):
    1 cycle/row on the PE at N>=256 vs 4 cycles/row for full fp32.
    Inputs are pre-rounded on the host so DMAs are pure copies.
  - Formulation: psum[r128, o512] = sum_k xT[k][:, r].T @ WT[k][:, o]
    accumulated over 8 K-tiles, plus ONE augmented K=17 matmul that adds
    both the LoRA rank-16 update and the bias:
       lhsT_aug = [ (g * (x @ A).T) ; ones ]  (17 x r)
       rhs_aug  = [ alpha * B_lora[-1] ; b ]  (17 x o)
    The gate is folded into the rank-16 intermediate (per-row scale
    commutes with the second LoRA matmul); alpha is folded into B.
  - PSUM->SBUF copies and the gate multiply run on the Vector engine.
    Weight blocks + small tensors stream on the SP HWDGE ring while xt
    chunks stream in parallel on the ACT ring (which later carries the
    output stores); the ob=0 row tiles are emitted interleaved with the
    LoRA matmuls per K-chunk so the PE static order is paced by xt-chunk
    arrivals instead of stalling on the full resident load.

Measured (8 cores, full inputs): relative error 1.7e-4 vs the fp32 jax
reference; per-core cost-model time 144.0 us (PE-bound; fp32r matmul
roofline for this decomposition is ~126 us/core).
"""

import numpy as np

import concourse.bacc as bacc
import concourse.mybir as mybir
import concourse.tile as tile
from concourse.bass_utils import run_bass_kernel_spmd

F32 = mybir.dt.float32
F32R = mybir.dt.float32r

N_CORES = 8
B, S, D_IN, D_OUT, R_LORA = 4, 2048, 1024, 4096, 16
ROWS = B * S                  # 8192
R_CORE = ROWS // N_CORES      # 1024 rows per core
KT = D_IN // 128              # 8 K-tiles of 128
NB = 512                      # moving free dim per matmul (max for 4-byte)
OB = D_OUT // NB              # 8 output blocks
RT = R_CORE // 128            # 8 row tiles per core
KA = R_LORA + 1               # augmented contraction (16 LoRA + 1 bias)


def _round_fp32r(a: np.ndarray) -> np.ndarray:
    """Round fp32 to the fp32r-representable set (11-bit mantissa,
    round-half-up in magnitude, carry into exponent OK)."""
    a = np.ascontiguousarray(a, dtype=np.float32)
    u = a.view(np.uint32)
    r = ((u + np.uint32(0x800)) & np.uint32(0xFFFFF000)).astype(np.uint32)
    return r.view(np.float32)


AUG_FIRST = True

def _build(
    xt_chunks: int = 8,
    xt_engine: str = "scalar",
    wt_bufs: int = 3,
    psum_bufs: int = 8,
    out_bufs: int = 4,
    wt0_split: int = 8,
    wt_split: int = 4,
    wt_alternate: bool = False,
):
    nc = bacc.Bacc(None, target_bir_lowering=False)

    x_d = nc.dram_tensor("xt", [128, KT, R_CORE], F32R, kind="ExternalInput")
    w_d = nc.dram_tensor("wt", [128, OB, KT, NB], F32R, kind="ExternalInput")
    a_d = nc.dram_tensor("a_lora", [128, KT, R_LORA], F32R, kind="ExternalInput")
    rhs_d = nc.dram_tensor("aug_rhs", [KA, D_OUT], F32R, kind="ExternalInput")
    g_d = nc.dram_tensor("g_rep", [R_LORA, R_CORE], F32, kind="ExternalInput")
    one_d = nc.dram_tensor("ones_row", [1, R_CORE], F32R, kind="ExternalInput")
    out_d = nc.dram_tensor("out", [R_CORE, D_OUT], F32, kind="ExternalOutput")

    with tile.TileContext(nc) as tc:
        with (
            tc.tile_pool(name="resident", bufs=1) as res,
            tc.tile_pool(name="wpool", bufs=wt_bufs) as wpool,
            tc.tile_pool(name="opool", bufs=out_bufs) as opool,
            tc.tile_pool(name="psum", bufs=psum_bufs, space="PSUM") as psum,
        ):
            # ---- resident loads -------------------------------------------------
            # Critical path at t=0 is (small tensors) + (wt block 0) on the SP
            # ring and xt chunk k on the ACT ring. wt block 0 is split along K
            # so the first matmuls unblock early.
            ld = getattr(nc, xt_engine)
            a_sb = res.tile([128, KT, R_LORA], F32R)
            nc.sync.dma_start(a_sb[:], a_d.ap())
            wt0 = wpool.tile([128, KT, NB], F32R, tag="wt")
            kh = KT // wt0_split
            for h in range(wt0_split):
                nc.sync.dma_start(
                    wt0[:, h * kh : (h + 1) * kh, :],
                    w_d.ap()[:, 0, h * kh : (h + 1) * kh, :],
                )
            # g/rhs/ones are not consumed until the gate multiply and first
            # aug matmul (~18us in) — load them after wt block 0
            g_sb = res.tile([R_LORA, R_CORE], F32)
            nc.sync.dma_start(g_sb[:], g_d.ap())
            rhs_sb = res.tile([KA, D_OUT], F32R)
            nc.sync.dma_start(rhs_sb[:], rhs_d.ap())
            lora_aug = res.tile([KA, R_CORE], F32R)
            nc.sync.dma_start(lora_aug[R_LORA : R_LORA + 1, :], one_d.ap())
            xt = res.tile([128, KT, R_CORE], F32R)
            if xt_chunks <= KT:
                kc = KT // xt_chunks
                for k in range(xt_chunks):
                    if k == 0 and kc == 1:
                        # split the first chunk in R-halves: the first lora +
                        # main matmuls unblock after 256KB instead of 512KB
                        hr = R_CORE // 2
                        ld.dma_start(xt[:, 0, 0:hr], x_d.ap()[:, 0, 0:hr])
                        ld.dma_start(xt[:, 0, hr:R_CORE], x_d.ap()[:, 0, hr:R_CORE])
                        continue
                    ld.dma_start(
                        xt[:, k * kc : (k + 1) * kc, :],
                        x_d.ap()[:, k * kc : (k + 1) * kc, :],
                    )
            else:
                rsplit = xt_chunks // KT
                rc = R_CORE // rsplit
                for k in range(KT):
                    for rh in range(rsplit):
                        ld.dma_start(
                            xt[:, k, rh * rc : (rh + 1) * rc],
                            x_d.ap()[:, k, rh * rc : (rh + 1) * rc],
                        )
            def emit_epilogue(ps, rt, ob):
                """Close psum tile: copy to SBUF, then store. For the last
                o-block, split copy+store in halves across both HWDGE rings
                (the SP ring is load-free by then) to shorten the tail chain."""
                o_sb = opool.tile([128, NB], F32, tag="o_sb", name=f"o_{ob}_{rt}")
                orow = out_d.ap()[rt * 128 : (rt + 1) * 128, ob * NB : (ob + 1) * NB]
                if ob == OB - 1:
                    h = NB // 2
                    nc.vector.tensor_copy(o_sb[:, 0:h], ps[:, 0:h])
                    nc.scalar.dma_start(orow[:, 0:h], o_sb[:, 0:h])
                    nc.vector.tensor_copy(o_sb[:, h:NB], ps[:, h:NB])
                    nc.sync.dma_start(orow[:, h:NB], o_sb[:, h:NB])
                else:
                    nc.vector.tensor_copy(o_sb[:], ps[:])
                    nc.scalar.dma_start(orow[:], o_sb[:])

            def emit_aug(ps, rt, ob, start, stop):
                # rank-16 LoRA update + bias in one K=17 matmul
                nc.tensor.matmul(
                    ps[:],
                    lora_aug[:, rt * 128 : (rt + 1) * 128],
                    rhs_sb[:, ob * NB : (ob + 1) * NB],
                    start=start,
                    stop=stop,
                )

            # ---- prologue: ob=0 interleaved with the LoRA first matmul ---------
            # PE static order is paced by xt-chunk arrivals, so per K-chunk we
            # emit the 2 lora matmuls plus 6 of the 8 ob=0 row tiles (2 lora +
            # 6 main psum tiles = 8 banks). rt=6,7 run densely afterwards.
            NRB = R_CORE // NB  # lora row blocks
            ps_l = [psum.tile([R_LORA, NB], F32, tag="ps", name=f"psl{rb}") for rb in range(NRB)]
            ps0 = [psum.tile([128, NB], F32, tag="ps", name=f"ps0_{rt}") for rt in range(6)]
            for k in range(KT):
                for rb in range(NRB):
                    nc.tensor.matmul(
                        ps_l[rb][:],
                        a_sb[:, k, :],
                        xt[:, k, rb * NB : (rb + 1) * NB],
                        start=(k == 0),
                        stop=(k == KT - 1),
                    )
                for rt in range(6):
                    nc.tensor.matmul(
                        ps0[rt][:],
                        xt[:, k, rt * 128 : (rt + 1) * 128],
                        wt0[:, k, :],
                        start=(k == 0),
                        stop=False,
                    )
            # gate multiply, rounded to fp32r for the augmented matmul
            for rb in range(NRB):
                nc.vector.tensor_mul(
                    lora_aug[0:R_LORA, rb * NB : (rb + 1) * NB],
                    ps_l[rb][:],
                    g_sb[:, rb * NB : (rb + 1) * NB],
                )
            for rt in range(6):
                emit_aug(ps0[rt], rt, 0, start=False, stop=True)
                emit_epilogue(ps0[rt], rt, 0)
            for rt in (6, 7):
                ps = psum.tile([128, NB], F32, tag="ps", name=f"ps0b_{rt}")
                for k in range(KT):
                    nc.tensor.matmul(
                        ps[:],
                        xt[:, k, rt * 128 : (rt + 1) * 128],
                        wt0[:, k, :],
                        start=(k == 0),
                        stop=False,
                    )
                emit_aug(ps, rt, 0, start=False, stop=True)
                emit_epilogue(ps, rt, 0)

            # ---- steady state: ob = 1..7 ---------------------------------------
            for ob in range(1, OB):
                wt = wpool.tile([128, KT, NB], F32R, tag="wt", name=f"wt{ob}")
                kw = KT // wt_split
                # alternate rings: the ACT ring is load-free once xt lands,
                # doubling weight delivery rate while the pipeline catches up
                wt_eng = nc.scalar if (ob % 2 == 1 and wt_alternate) else nc.sync
                for h in range(wt_split):
                    wt_eng.dma_start(
                        wt[:, h * kw : (h + 1) * kw, :],
                        w_d.ap()[:, ob, h * kw : (h + 1) * kw, :],
                    )
                for rt in range(RT):
                    ps = psum.tile([128, NB], F32, tag="ps", name=f"ps{ob}_{rt}")
                    if AUG_FIRST:
                        emit_aug(ps, rt, ob, start=True, stop=False)
                    for k in range(KT):
                        nc.tensor.matmul(
                            ps[:],
                            xt[:, k, rt * 128 : (rt + 1) * 128],
                            wt[:, k, :],
                            start=(not AUG_FIRST and k == 0),
                            stop=(AUG_FIRST and k == KT - 1),
                        )
                    if not AUG_FIRST:
                        emit_aug(ps, rt, ob, start=False, stop=True)
                    emit_epilogue(ps, rt, ob)

    nc.compile()
    return nc


_NC_CACHE = None


def _get_nc():
    global _NC_CACHE
    if _NC_CACHE is None:
        _NC_CACHE = _build()
    return _NC_CACHE


def _prep_in_maps(x, W, b, A, B_lora, gates, alpha):
    x = np.asarray(x, dtype=np.float32).reshape(ROWS, D_IN)
    W = np.asarray(W, dtype=np.float32)
    b = np.asarray(b, dtype=np.float32)
    A_last = np.asarray(A, dtype=np.float32)[-1]          # [D_IN, 16]
    B_last = np.asarray(B_lora, dtype=np.float32)[-1]     # [16, D_OUT]
    g_last = np.asarray(gates, dtype=np.float32)[-1].reshape(ROWS)
    alpha_f = float(np.asarray(alpha))

    # W.T packed as [ki, ob, ko, o'] so each o-block DMA is one contiguous
    # 16 KiB run per partition.
    wt = W.T.reshape(KT, 128, OB, NB).transpose(1, 2, 0, 3)
    w_pre = _round_fp32r(np.ascontiguousarray(wt))

    a_pre = _round_fp32r(
        np.ascontiguousarray(A_last.reshape(KT, 128, R_LORA).transpose(1, 0, 2))
    )
    aug = np.concatenate([alpha_f * B_last, b[None, :]], axis=0)  # [17, D_OUT]
    aug_pre = _round_fp32r(aug)
    ones_row = _round_fp32r(np.ones((1, R_CORE), dtype=np.float32))

    in_maps = []
    for c in range(N_CORES):
        rows = slice(c * R_CORE, (c + 1) * R_CORE)
        xs = x[rows]                                      # [R_CORE, D_IN]
        xt = xs.T.reshape(KT, 128, R_CORE).transpose(1, 0, 2)
        x_pre = _round_fp32r(np.ascontiguousarray(xt))
        g_rep = np.ascontiguousarray(
            np.broadcast_to(g_last[rows][None, :], (R_LORA, R_CORE))
        ).astype(np.float32)
        in_maps.append(
            {
                "xt": x_pre,
                "wt": w_pre,
                "a_lora": a_pre,
                "aug_rhs": aug_pre,
                "g_rep": g_rep,
                "ones_row": ones_row,
            }
        )
    return in_maps


def run(inputs: dict, trace: bool = False, trace_cores=None):
    """Run the kernel; returns (full_output, BassKernelResults)."""
    nc = _get_nc()
    in_maps = _prep_in_maps(**inputs)
    res = run_bass_kernel_spmd(
        nc,
        in_maps,
        core_ids=list(range(N_CORES)),
        trace=trace,
        trace_cores=trace_cores,
    )
    out = np.concatenate([r["out"] for r in res.results], axis=0)
    return out.reshape(B, S, D_OUT).astype(np.float32), res


def kernel(**inputs) -> np.ndarray:
    out, _ = run(inputs, trace=False)
    return out



# revision 15
# speedup vs baseline: 1.0599x; 1.0599x over previous
"""Trainium2 Bass kernel for ClassLinearWithLORA (moe_routing).

Computes out = x @ W.T + b + gates[-1] * (alpha * (x @ A[-1]) @ B_lora[-1])
(the torch loop overwrites out_lora each class iteration, so only the last
class adapter contributes).

Strategy:
  - Data-parallel shard of the 8192 (B*S) rows across 8 NeuronCores
    (1024 rows/core); W/b and the rank-16 LoRA stacks are replicated.
  - Matmuls run in bf16 (1 cycle/row on the PE, same rate as fp32r, but
    half the DMA bytes): rel err ~3e-3 vs the fp32 reference, well under
    the 2e-2 gate.  PSUM accumulation and the output stay fp32.
  - Formulation: psum[r128, o512] = sum_k xT[k][:, r].T @ WT[k][:, o]
    accumulated over 8 K-tiles, plus ONE augmented K=17 matmul per tile that
    adds both the LoRA rank-16 update and the bias:
       lhsT_aug = [ (g * (x @ A)).T ; ones ]  (17 x r)
       rhs_aug  = [ alpha * B_lora[-1] ; b ]  (17 x o)
  - LoRA stage 1 (x @ A) runs with the rank-16 side as the MOVING dim
    (ap=16 -> ~7ns/matmul instead of 213ns), accumulating all 8 row tiles
    into one PSUM bank; the gate is applied by ONE DVE multiply against a
    host-replicated [128, 128] gate tile, and PE transposes (via an
    identity tile) flip the [128,16] blocks into the [16,128] layout the
    aug matmul needs.
  - Startup: dummy matmuls on a memset tile keep the PE busy from ~0.4us so
    the p-state ramp (half clock for the first 3us of PE busy) is burned
    before real data lands; loads are interleaved across the two HWDGE
    rings in need-order (each DMA costs ~630ns of the shared
    descriptor-gen stage, which dominates the startup critical path).
  - Epilogues: PSUM->SBUF copies alternate DVE/ACT, stores alternate the
    two HWDGE rings.  The final row tile of the last output block is four
    independent [128,128] PSUM groups emitted so that only the smallest
    possible chain (one copy + one store on an idle ring) follows the last
    matmul.
"""

import numpy as np
import ml_dtypes

import concourse.bacc as bacc
import concourse.mybir as mybir
import concourse.tile as tile
from concourse.bass_utils import run_bass_kernel_spmd

F32 = mybir.dt.float32
BF16 = mybir.dt.bfloat16

N_CORES = 8
B, S, D_IN, D_OUT, R_LORA = 4, 2048, 1024, 4096, 16
ROWS = B * S                  # 8192
R_CORE = ROWS // N_CORES      # 1024 rows per core
KT = D_IN // 128              # 8 K-tiles of 128
NB = 512                      # moving free dim per matmul
OB = D_OUT // NB              # 8 output blocks
RT = R_CORE // 128            # 8 row tiles per core
KA = R_LORA + 1               # augmented contraction (16 LoRA + 1 bias)

BF = ml_dtypes.bfloat16


def _build(
    warmup=(512, 512, 256, 256, 128, 128, 64, 64),
    tail_split: int = 4,
    wt_bufs: int = 3,
    out_bufs: int = 10,
):
    nc = bacc.Bacc(None, target_bir_lowering=False)

    x_d = nc.dram_tensor("xt", [128, KT, R_CORE], BF16, kind="ExternalInput")
    w_d = nc.dram_tensor("wt", [128, OB, KT, NB], BF16, kind="ExternalInput")
    a_d = nc.dram_tensor("a_lora", [128, KT, R_LORA], BF16, kind="ExternalInput")
    rhs_d = nc.dram_tensor("aug_rhs", [KA, D_OUT], BF16, kind="ExternalInput")
    g_d = nc.dram_tensor("g_full", [128, RT * R_LORA], F32, kind="ExternalInput")
    id_d = nc.dram_tensor("ident", [128, 128], BF16, kind="ExternalInput")
    one_d = nc.dram_tensor("ones_row", [1, R_CORE], BF16, kind="ExternalInput")
    out_d = nc.dram_tensor("out", [R_CORE, D_OUT], F32, kind="ExternalOutput")

    with tile.TileContext(nc) as tc:
        with (
            tc.tile_pool(name="resident", bufs=1) as res,
            tc.tile_pool(name="wpool", bufs=wt_bufs) as wpool,
            tc.tile_pool(name="opool", bufs=out_bufs) as opool,
            tc.tile_pool(name="psum", bufs=6, space="PSUM") as psum,
            tc.tile_pool(name="psum2", bufs=2, space="PSUM") as psum2,
        ):
            # ---- PE warmup: burn the p-state ramp on a memset tile -------------
            if warmup:
                dum = res.tile([128, 512], BF16)
                nc.vector.memset(dum[:], 0.0)
                dps = psum2.tile([16, 512], F32, tag="ps2", name="dummy")
                for ap in warmup:
                    nc.tensor.matmul(
                        dps[:, 0:ap], dum[:, 0:16], dum[:, 0:ap],
                        start=True, stop=True,
                    )

            # ---- resident loads, interleaved across rings in need-order --------
            wt0 = wpool.tile([128, KT, NB], BF16, tag="wt")
            xt = res.tile([128, KT, R_CORE], BF16)
            a_sb = res.tile([128, KT, R_LORA], BF16)
            g_sb = res.tile([128, RT * R_LORA], F32)
            id_sb = res.tile([128, 128], BF16)
            rhs_sb = res.tile([KA, D_OUT], BF16)
            lora_aug = res.tile([KA, R_CORE], BF16)

            nc.sync.dma_start(xt[:, 0, :], x_d.ap()[:, 0, :])
            nc.scalar.dma_start(wt0[:, 0:1, :], w_d.ap()[:, 0, 0:1, :])
            nc.sync.dma_start(wt0[:, 1:3, :], w_d.ap()[:, 0, 1:3, :])
            nc.scalar.dma_start(xt[:, 1, :], x_d.ap()[:, 1, :])
            nc.sync.dma_start(a_sb[:], a_d.ap())
            nc.scalar.dma_start(xt[:, 2, :], x_d.ap()[:, 2, :])
            nc.sync.dma_start(wt0[:, 3:6, :], w_d.ap()[:, 0, 3:6, :])
            nc.scalar.dma_start(xt[:, 3, :], x_d.ap()[:, 3, :])
            nc.sync.dma_start(wt0[:, 6:KT, :], w_d.ap()[:, 0, 6:KT, :])
            nc.scalar.dma_start(xt[:, 4, :], x_d.ap()[:, 4, :])
            nc.sync.dma_start(g_sb[:], g_d.ap())
            nc.scalar.dma_start(xt[:, 5, :], x_d.ap()[:, 5, :])
            nc.sync.dma_start(id_sb[:], id_d.ap())
            nc.scalar.dma_start(xt[:, 6, :], x_d.ap()[:, 6, :])
            nc.sync.dma_start(rhs_sb[:], rhs_d.ap())
            nc.scalar.dma_start(xt[:, 7, :], x_d.ap()[:, 7, :])
            nc.scalar.dma_start(lora_aug[R_LORA : R_LORA + 1, :], one_d.ap())

            def emit_epilogue(ps, rt, ob, col0=0, ncol=NB, cp=None, st=None):
                """Close a psum region: copy to SBUF (DVE or ACT), then store
                (either HWDGE ring)."""
                o_sb = opool.tile([128, NB], F32, tag="o_sb", name=f"o_{ob}_{rt}_{col0}")
                orow = out_d.ap()[
                    rt * 128 : (rt + 1) * 128,
                    ob * NB + col0 : ob * NB + col0 + ncol,
                ]
                st = st or (nc.scalar if rt % 2 == 0 else nc.sync)
                nc.vector.tensor_copy(o_sb[:, col0 : col0 + ncol], ps[:, 0:ncol])
                st.dma_start(orow[:], o_sb[:, col0 : col0 + ncol])

            def emit_aug(ps, rt, ob, start, stop, col0=0, ncol=NB):
                # rank-16 LoRA update + bias in one K=17 matmul
                nc.tensor.matmul(
                    ps[:, 0:ncol],
                    lora_aug[:, rt * 128 : (rt + 1) * 128],
                    rhs_sb[:, ob * NB + col0 : ob * NB + col0 + ncol],
                    start=start,
                    stop=stop,
                )

            def emit_mains(ps, rt, ob, wt, col0=0, ncol=NB):
                for k in range(KT):
                    nc.tensor.matmul(
                        ps[:, 0:ncol],
                        xt[:, k, rt * 128 : (rt + 1) * 128],
                        wt[:, k, col0 : col0 + ncol],
                        start=False,
                        stop=(k == KT - 1),
                    )

            # ---- prologue: ob=0 (rt0-5) interleaved with LoRA stage 1 ----------
            # x@A with the rank-16 side moving: 64 matmuls of ap=16 into one
            # PSUM bank (slices per row tile), ~0.5us of PE instead of 3.4us.
            ps_xa = psum2.tile([128, RT * R_LORA], F32, tag="ps2", name="ps_xa")
            ps0 = [psum.tile([128, NB], F32, tag="ps", name=f"ps0_{rt}") for rt in range(6)]
            def _xa(k):
                for rt in range(RT):
                    # start=True zeroes the WHOLE 2KB psum bank, so only the
                    # very first matmul of the shared-bank tile may set it;
                    # later slices accumulate onto the zeroed region.
                    nc.tensor.matmul(
                        ps_xa[:, rt * R_LORA : (rt + 1) * R_LORA],
                        xt[:, k, rt * 128 : (rt + 1) * 128],
                        a_sb[:, k, :],
                        start=(k == 0 and rt == 0),
                        stop=(k == KT - 1 and rt == RT - 1),
                    )
            for k in range(KT):
                # xa trails the mains by 2 K-steps so a_sb is off the startup
                # critical path; at k=KT-1 all remaining xa passes run BEFORE
                # the mains so the gate/transpose chain overlaps them
                if k == KT - 1:
                    _xa(KT - 3)
                    _xa(KT - 2)
                    _xa(KT - 1)
                elif k >= 2:
                    _xa(k - 2)
                for rt in range(6):
                    nc.tensor.matmul(
                        ps0[rt][:],
                        xt[:, k, rt * 128 : (rt + 1) * 128],
                        wt0[:, k, :],
                        start=(k == 0),
                        stop=False,
                    )

            # gate: one DVE multiply against the host-replicated gate tile
            xa_g = res.tile([128, RT * R_LORA], BF16)
            nc.vector.tensor_mul(xa_g[:], ps_xa[:], g_sb[:])
            # transpose each 128x16 block to 16x128 through the PE
            ps_t = [
                psum2.tile([R_LORA, 512], BF16, tag="ps2", name=f"ps_t{h}")
                for h in range(2)
            ]
            for rt in range(RT):
                nc.tensor.transpose(
                    ps_t[rt // 4][:, (rt % 4) * 128 : (rt % 4 + 1) * 128],
                    xa_g[:, rt * R_LORA : (rt + 1) * R_LORA],
                    id_sb[:],
                )
            # two 512-wide copies into the aug lhsT (frees each ps_t tile
            # with a single DVE op)
            for h in range(2):
                nc.vector.tensor_copy(
                    lora_aug[0:R_LORA, h * 512 : (h + 1) * 512], ps_t[h][:]
                )

            # rt6/rt7 dense blocks take the freed psum2 slots so they don't
            # wait on the rt0-5 epilogue copies; their mains run start-first
            # (no lora dependency) with the aug last, hiding the gate/
            # transpose/copy chain behind 3.4us of PE work.
            def emit_mains_first(ps, rt, wt):
                for k in range(KT):
                    nc.tensor.matmul(
                        ps[:],
                        xt[:, k, rt * 128 : (rt + 1) * 128],
                        wt[:, k, :],
                        start=(k == 0),
                        stop=False,
                    )
            ps6 = psum2.tile([128, NB], F32, tag="ps2", name="ps0b_6")
            emit_mains_first(ps6, 6, wt0)
            for rt in range(3):
                emit_aug(ps0[rt], rt, 0, start=False, stop=True)
                emit_epilogue(ps0[rt], rt, 0)
            ps7 = psum2.tile([128, NB], F32, tag="ps2", name="ps0b_7")
            emit_mains_first(ps7, 7, wt0)
            for rt in range(3, 6):
                emit_aug(ps0[rt], rt, 0, start=False, stop=True)
                emit_epilogue(ps0[rt], rt, 0)
            emit_aug(ps6, 6, 0, start=False, stop=True)
            emit_epilogue(ps6, 6, 0)
            emit_aug(ps7, 7, 0, start=False, stop=True)
            emit_epilogue(ps7, 7, 0)

            # ---- steady state: ob = 1..7 ---------------------------------------
            for ob in range(1, OB):
                wt = wpool.tile([128, KT, NB], BF16, tag="wt", name=f"wt{ob}")
                nc.sync.dma_start(wt[:, 0:4, :], w_d.ap()[:, ob, 0:4, :])
                nc.sync.dma_start(wt[:, 4:KT, :], w_d.ap()[:, ob, 4:KT, :])
                last_ob = ob == OB - 1
                if not last_ob:
                    for rt in range(RT):
                        ps = psum.tile([128, NB], F32, tag="ps", name=f"ps{ob}_{rt}")
                        emit_aug(ps, rt, ob, start=True, stop=False)
                        emit_mains(ps, rt, ob, wt)
                        emit_epilogue(ps, rt, ob)
                else:
                    # last block: rt7 runs as tail_split independent column
                    # groups; all but the last are emitted before rt6 so only
                    # one small copy+store chain follows the final matmul.
                    nsub = NB // tail_split
                    for rt in range(6):
                        ps = psum.tile([128, NB], F32, tag="ps", name=f"ps{ob}_{rt}")
                        emit_aug(ps, rt, ob, start=True, stop=False)
                        emit_mains(ps, rt, ob, wt)
                        emit_epilogue(ps, rt, ob)
                    ncol_last = nsub
                    ncol_0 = NB - ncol_last
                    ps6l = psum.tile([128, NB], F32, tag="ps", name=f"ps{ob}_6")
                    emit_aug(ps6l, 6, ob, start=True, stop=False)
                    emit_mains(ps6l, 6, ob, wt)
                    psa = psum.tile([128, ncol_0], F32, tag="ps", name="pst_0")
                    emit_aug(psa, 7, ob, start=True, stop=False,
                             col0=0, ncol=ncol_0)
                    emit_mains(psa, 7, ob, wt, col0=0, ncol=ncol_0)
                    emit_epilogue(ps6l, 6, ob, st=nc.sync)
                    psb = psum.tile([128, ncol_last], F32, tag="ps", name="pst_1")
                    emit_aug(psb, 7, ob, start=True, stop=False,
                             col0=ncol_0, ncol=ncol_last)
                    emit_mains(psb, 7, ob, wt, col0=ncol_0, ncol=ncol_last)
                    emit_epilogue(psa, 7, ob, col0=0, ncol=ncol_0, st=nc.sync)
                    emit_epilogue(psb, 7, ob, col0=ncol_0, ncol=ncol_last,
                                  st=nc.scalar)

    nc.compile()
    return nc


_NC_CACHE = None


def _get_nc():
    global _NC_CACHE
    if _NC_CACHE is None:
        _NC_CACHE = _build()
    return _NC_CACHE


def _prep_in_maps(x, W, b, A, B_lora, gates, alpha):
    x = np.asarray(x, dtype=np.float32).reshape(ROWS, D_IN)
    W = np.asarray(W, dtype=np.float32)
    b = np.asarray(b, dtype=np.float32)
    A_last = np.asarray(A, dtype=np.float32)[-1]          # [D_IN, 16]
    B_last = np.asarray(B_lora, dtype=np.float32)[-1]     # [16, D_OUT]
    g_last = np.asarray(gates, dtype=np.float32)[-1].reshape(ROWS)
    alpha_f = float(np.asarray(alpha))

    # W.T packed as [ki, ob, ko, o'] so each o-block DMA is one contiguous
    # run per partition.
    wt = W.T.reshape(KT, 128, OB, NB).transpose(1, 2, 0, 3)
    w_pre = np.ascontiguousarray(wt).astype(BF)

    a_pre = np.ascontiguousarray(
        A_last.reshape(KT, 128, R_LORA).transpose(1, 0, 2)
    ).astype(BF)
    aug = np.concatenate([alpha_f * B_last, b[None, :]], axis=0)  # [17, D_OUT]
    aug_pre = aug.astype(BF)
    ones_row = np.ones((1, R_CORE), dtype=BF)
    ident = np.eye(128, dtype=np.float32).astype(BF)

    in_maps = []
    for c in range(N_CORES):
        rows = slice(c * R_CORE, (c + 1) * R_CORE)
        xs = x[rows]                                      # [R_CORE, D_IN]
        xt = xs.T.reshape(KT, 128, R_CORE).transpose(1, 0, 2)
        x_pre = np.ascontiguousarray(xt).astype(BF)
        # g_full[p, rt*16 + r] = gate for row rt*128 + p of this shard
        g_full = np.ascontiguousarray(
            np.repeat(
                g_last[rows].reshape(RT, 128).T[:, :, None], R_LORA, axis=2
            ).reshape(128, RT * R_LORA)
        ).astype(np.float32)
        in_maps.append(
            {
                "xt": x_pre,
                "wt": w_pre,
                "a_lora": a_pre,
                "aug_rhs": aug_pre,
                "ones_row": ones_row,
                "g_full": g_full,
                "ident": ident,
            }
        )
    return in_maps


def run(inputs: dict, trace: bool = False, trace_cores=None):
    """Run the kernel; returns (full_output, BassKernelResults)."""
    nc = _get_nc()
    in_maps = _prep_in_maps(**inputs)
    res = run_bass_kernel_spmd(
        nc,
        in_maps,
        core_ids=list(range(N_CORES)),
        trace=trace,
        trace_cores=trace_cores,
    )
    out = np.concatenate([r["out"] for r in res.results], axis=0)
    return out.reshape(B, S, D_OUT).astype(np.float32), res


def kernel(**inputs) -> np.ndarray:
    out, _ = run(inputs, trace=False)
    return out
